# revision 1
# baseline (speedup 1.0000x reference)
"""Trainium2 Bass kernel for nn_AttentionBetweenWordsAndChars.

Reference computation (per batch b, word w):
    q/k/v projections of word_vec and char_vec (shared weights per proj),
    2x2 attention between the two representations, output [B, W, 2H].

Math (exact reformulation):
  - softmax over 2 keys == sigmoid of the logit difference:
        out_q = Vc + sigmoid(d_q) * Dv,   q in {word, char}
    with Dv = Vw - Vc, d_q = s(q,word) - s(q,char) = x~_q A Dx~^T,
    A = W~q W~k^T / sqrt(H)  (bias-augmented; computed host-side from the
    weights once, like any weight folding), Dx~ = x~w - x~c.
  - per 128-token tile: 9 matmuls (Vc, Dv: 3 k-chunks x N=512; g = Dx A^T:
    3 k-chunks x N=301), two length-301 row-dots for (d0, d1), one sigmoid,
    two scalar*tensor+tensor combines.

Precision plan (tolerance 2e-2; measured on host emulation):
  - matmul inputs bf16, fp32 PSUM accumulation  (~0.3% error)
  - token-major x for the row-dots in fp8-e4m3  (+~0.4%)
  - output stored bf16, upcast to fp32 on host  (+~0.06%)

DMA plan: HWDGE costs ~625ns per DMA regardless of size, so tiles are
grouped 4-per-DMA. One uint8 macro tile per group carries, per 128-token
sub-tile: 1536B of feature-major x~^T (bf16, zero-padded 45-row chunk) and
608B of token-major x~ (fp8), bitcast into typed views on SBUF. Output is
one grouped [128, 4x1024] bf16 DMA. Inputs issue on SP, outputs on ACT
(separate queues avoid head-of-line blocking of the prefetch stream).

Engine balance per tile (cost-model): PE 1.66us (critical), DVE ~1.65us
(sub + dot0 + combine0), POOL ~1.4us (dot1 + combine1), ACT ~1.2us
(g evac + sigmoid + out-DMA issue).

Sharding: data-parallel over batch; each of the 8 cores gets 8 batches
(4096 tokens). Weights replicated. No collectives.
"""

import sys

for _p in ("/opt/trn_rl_repo", "/root/.axon_site/_ro/trn_rl_repo"):
    if _p not in sys.path:
        sys.path.insert(0, _p)

import numpy as np

import concourse.bass as bass
import concourse.tile as tile
from concourse import mybir
from concourse.bass_utils import run_bass_kernel_spmd
import bass_rust

B, W, D_IN, H = 64, 512, 300, 512
N_CORES = 8
BPC = B // N_CORES                # batches per core
TOK = BPC * W                     # 4096 tokens per core
TILES = TOK // 128                # 32
GRP = 4                           # tiles per DMA group
NGROUPS = TILES // GRP            # 8
DA = D_IN + 1                     # 301 augmented dim
TEMP = float(np.sqrt(np.float32(H)))
F32 = mybir.dt.float32
BF16 = mybir.dt.bfloat16
FP8 = mybir.dt.float8e4
U8 = mybir.dt.uint8
AF = mybir.ActivationFunctionType
OP = mybir.AluOpType

# contraction chunks of 301 sized so the feature-major feed needs almost no
# zero padding (rows 99/100 of chunk 2 only)
CHUNKS = [(0, 101), (101, 101), (202, 99)]
XT_ROWS = 101                     # partition rows of the x~^T feed
XT_COLS = 768                     # bf16 cols per sub-tile (2 sides x 3 x 128)
X8_COLS = 608                     # fp8 token-major bytes per sub-tile


def spill_excess_waits(nc, max_keep=1, ev_cap=2):
    """walrus accepts very few sync-wait commands per instruction (1 for
    most datapath opcodes). Move excess waits onto pure-wait EventSemaphore
    instructions inserted immediately before the offender on the same
    engine queue — semantically identical (FIFO queue), encoding-legal."""
    counter = 0
    for f in nc.m.functions:
        for blk in f.blocks:
            insts = blk.instructions
            i = 0
            while i < len(insts):
                inst = insts[i]
                si = inst.sync_info
                if si is None:
                    i += 1
                    continue
                w = list(si.on_wait or [])
                if len(w) > max_keep:
                    spill = w[:-max_keep]
                    for j in range(0, len(spill), ev_cap):
                        ev = mybir.InstEventSemaphore(name=f"wspill_{counter}")
                        counter += 1
                        ev.engine = inst.engine
                        ev.sync_info = bass_rust.SyncInfo(
                            on_wait=spill[j:j + ev_cap], on_update=[]
                        )
                        insts.insert(i, ev)
                        i += 1
                    inst.sync_info.on_wait = w[-max_keep:]
                i += 1
    return counter


def build_program(loop_reps=1, ab_no_logits=False, ab_x8_bf16=False,
                  ab_no_m1=False, ab_no_dma_in=False, ab_no_dma_out=False,
                  ab_mm_only=False, ab_noskew=False, ab_pertile_in=False):
    nc = bass.Bass("TRN2", target_bir_lowering=False, debug=False,
                   num_devices=N_CORES)
    xt_d = nc.dram_tensor("xt", [NGROUPS * XT_ROWS, GRP * XT_COLS], BF16,
                          kind="ExternalInput").ap()
    x8_dt = BF16 if ab_x8_bf16 else FP8
    x8_d = nc.dram_tensor("x8", [NGROUPS * 128, GRP * X8_COLS], x8_dt,
                          kind="ExternalInput").ap()
    # weights packed 128-partition: per k-chunk c, A^T chunk at cols c*304
    # (301 valid) and W~v chunk at cols c*512 of the second tensor
    wat_d = nc.dram_tensor("Wat", [128, 3 * 304], BF16,
                           kind="ExternalInput").ap()
    wwv_d = nc.dram_tensor("Wwv", [128, 3 * 512], BF16,
                           kind="ExternalInput").ap()
    out_d = nc.dram_tensor("out", [TOK, 2 * H], BF16,
                           kind="ExternalOutput").ap()

    from contextlib import ExitStack

    with tile.TileContext(nc) as tc, ExitStack() as es:
        cpool = es.enter_context(tc.tile_pool(name="consts", bufs=1))
        # Engine-assignment constraints from the walrus birverifier:
        #  - GPSIMD (Pool) cannot touch PSUM and has no TensorScalarPtr
        #  - any DVE/ACT op may read at most ONE non-scalar PSUM operand
        # Split: POOL does the sub (TensorTensor, SBUF-only). ACT evacuates
        # g and vc to SBUF, computes m1 = a1*Dv via a scale-AP copy, and the
        # sigmoid. DVE does both row-dots, combine0 as STT (dv from PSUM,
        # vc from SBUF) and combine1 as a bf16 TensorTensor add (m1 + vc).
        ps_g = es.enter_context(tc.tile_pool(name="ps_g", bufs=2,
                                             space="PSUM"))
        ps_vc = es.enter_context(tc.tile_pool(name="ps_vc", bufs=2,
                                              space="PSUM"))
        ps_dv = es.enter_context(tc.tile_pool(name="ps_dv", bufs=4,
                                              space="PSUM"))

        zbias = cpool.tile([128, 1], F32, tag="zbias")
        nc.gpsimd.memset(zbias[:], 0.0)
        # preload the sigmoid ACT table set while the weights load
        sigwarm = cpool.tile([128, 1], F32, tag="sigwarm")
        nc.scalar.activation(sigwarm[:], zbias[:], AF.Sigmoid, bias=zbias[:])

        # resident weights: two single-DMA tiles (HWDGE cost is per-DMA)
        att = cpool.tile([128, 3 * 304], BF16, tag="wat", name="wat")
        wvt = cpool.tile([128, 3 * 512], BF16, tag="wwv", name="wwv")
        at_b = [att[0:sz, c * 304:c * 304 + DA]
                for c, (_, sz) in enumerate(CHUNKS)]
        wv_b = [wvt[0:sz, c * 512:(c + 1) * 512]
                for c, (_, sz) in enumerate(CHUNKS)]

        pin = es.enter_context(tc.tile_pool(name="pin", bufs=3))
        pdx = es.enter_context(tc.tile_pool(name="pdx", bufs=2 * GRP))
        psc = es.enter_context(tc.tile_pool(name="psc", bufs=6))
        pout = es.enter_context(tc.tile_pool(name="pout", bufs=6))

        NG = NGROUPS * loop_reps

        def fetch_group(g):
            """DMA one input group and return its sub-tile views."""
            xtg = pin.tile([128, GRP * XT_COLS], BF16, tag="xtg")
            x8g = pin.tile([128, GRP * X8_COLS], x8_dt, tag="x8g")
            if ab_pertile_in:
                for s in range(GRP):
                    nc.sync.dma_start(
                        xtg[0:XT_ROWS, s * XT_COLS:(s + 1) * XT_COLS],
                        xt_d[g * XT_ROWS:(g + 1) * XT_ROWS,
                             s * XT_COLS:(s + 1) * XT_COLS])
                    nc.sync.dma_start(
                        x8g[:, s * X8_COLS:(s + 1) * X8_COLS],
                        x8_d[g * 128:(g + 1) * 128,
                             s * X8_COLS:(s + 1) * X8_COLS])
            elif not ab_no_dma_in:
                nc.sync.dma_start(xtg[0:XT_ROWS, :],
                                  xt_d[g * XT_ROWS:(g + 1) * XT_ROWS, :])
                nc.sync.dma_start(x8g[:], x8_d[g * 128:(g + 1) * 128, :])
            return [(xtg[:, s * XT_COLS:(s + 1) * XT_COLS],
                     x8g[:, s * X8_COLS:(s + 1) * X8_COLS])
                    for s in range(GRP)]

        def emit_sub(xT, eng=None):
            """Dx~^T = x~w^T - x~c^T on POOL (SBUF-only engine)."""
            dx = pdx.tile([128, 384], BF16, tag="dx")
            (eng or nc.gpsimd).tensor_sub(dx[0:XT_ROWS, :],
                                          xT[0:XT_ROWS, 0:384],
                                          xT[0:XT_ROWS, 384:768])
            return dx

        def emit_subs(views, eng=None):
            return [emit_sub(views[s][0], eng) for s in range(GRP)]

        # software-pipelined group body: per-engine stage skew so no queue
        # head ever waits on a dependency whose producer sits behind it
        def emit_group(g, views, dxs, next_views):
            gsb = [None] * GRP
            vcsb = [None] * GRP
            dd = [None] * GRP
            sig = [None] * GRP
            next_dxs = None

            def emit_mms(s):
                xT, _ = views[s]
                dx = dxs[s]
                vc_ps = ps_vc.tile([128, H], F32, tag="vc")
                g_ps = ps_g.tile([128, 304], F32, tag="g")
                dv_ps = ps_dv.tile([128, H], F32, tag="dv")
                xcT = [xT[0:101, 384:512], xT[0:101, 512:640],
                       xT[0:99, 640:768]]
                dxT = [dx[0:101, 0:128], dx[0:101, 128:256],
                       dx[0:99, 256:384]]
                # g first: the logits chain is the long pole; vc next so its
                # ACT evacuation can free the bank while dv still runs
                for c in range(3):
                    nc.tensor.matmul(g_ps[:, 0:DA], dxT[c], at_b[c][:],
                                     start=(c == 0), stop=(c == 2))
                for c in range(3):
                    nc.tensor.matmul(vc_ps[:], xcT[c], wv_b[c][:],
                                     start=(c == 0), stop=(c == 2))
                for c in range(3):
                    nc.tensor.matmul(dv_ps[:], dxT[c], wv_b[c][:],
                                     start=(c == 0), stop=(c == 2))
                return g_ps, vc_ps, dv_ps

            def emit_evac_g(s):
                g_sb = psc.tile([128, 304], BF16, tag="gsb")
                nc.scalar.copy(g_sb[:, 0:DA], mm[s][0][:, 0:DA])
                return g_sb

            def emit_evac_vc(s):
                vc_sb = psc.tile([128, H], BF16, tag="vcsb")
                nc.scalar.copy(vc_sb[:], mm[s][1][:])
                return vc_sb

            def emit_dots(s):
                g_sb = gsb[s]
                _, x8 = views[s]
                ddt = psc.tile([128, 2], F32, tag="dd")
                sc0 = psc.tile([128, 304], BF16, tag="sc0")
                nc.vector.scalar_tensor_tensor(
                    out=sc0[:, 0:DA], in0=g_sb[:, 0:DA], scalar=1.0,
                    in1=x8[:, 0:DA], op0=OP.mult, op1=OP.mult,
                    accum_out=ddt[:, 0:1])
                sc1 = psc.tile([128, 304], BF16, tag="sc1")
                nc.vector.scalar_tensor_tensor(
                    out=sc1[:, 0:DA], in0=g_sb[:, 0:DA], scalar=1.0,
                    in1=x8[:, 304:304 + DA], op0=OP.mult, op1=OP.mult,
                    accum_out=ddt[:, 1:2])
                return ddt

            def emit_sig(s):
                aa = psc.tile([128, 2], F32, tag="aa")
                nc.scalar.activation(aa[:], dd[s][:], AF.Sigmoid,
                                     bias=zbias[:])
                if ab_no_m1:
                    return aa, None
                # m1 = a1 * Dv on ACT (scale-AP copy, the one PSUM operand)
                m1 = psc.tile([128, H], BF16, tag="m1")
                nc.scalar.mul(m1[:], mm[s][2][:], aa[:, 1:2])
                return aa, m1

            def emit_comb(s, last=False):
                _, _, dv_ps = mm[s]
                vc_sb = vcsb[s]
                aa, m1 = sig[s] if sig[s] is not None else (None, None)
                out_t = pout.tile([128, 2 * H], BF16, tag="out")
                sc_a = 0.5 if aa is None else aa[:, 0:1]
                nc.vector.scalar_tensor_tensor(
                    out=out_t[:, 0:H], in0=dv_ps[:],
                    scalar=sc_a, in1=vc_sb[:],
                    op0=OP.mult, op1=OP.add)
                if m1 is None:
                    sc_b = 0.5 if aa is None else aa[:, 1:2]
                    nc.vector.scalar_tensor_tensor(
                        out=out_t[:, H:2 * H], in0=dv_ps[:],
                        scalar=sc_b, in1=vc_sb[:],
                        op0=OP.mult, op1=OP.add)
                else:
                    # during drain the DVE is the backlog: POOL gets the adds
                    addeng = nc.gpsimd if last else nc.vector
                    addeng.tensor_add(out_t[:, H:2 * H], m1[:], vc_sb[:])
                r0 = (g * GRP + s) * 128
                if not ab_no_dma_out:
                    nc.sync.dma_start(out_d[r0:r0 + 128, :], out_t[:])

            mm = [None] * GRP
            # skewed stage schedule: dots+sigmoid one column behind their
            # matmuls, combines two behind (one behind for the final group,
            # which has no successor to starve — shortens the drain tail).
            # One next-group sub per column, emitted last so it never delays
            # dots/combines.
            last = next_views is None
            coff = 1 if last else 2
            next_dxs = [] if next_views is not None else None
            if ab_mm_only:
                for k in range(GRP):
                    mm[k] = emit_mms(k)
                    if next_views is not None:
                        next_dxs.append(emit_sub(next_views[k][0]))
                return next_dxs
            if ab_noskew:
                for k in range(GRP):
                    mm[k] = emit_mms(k)
                    gsb[k] = emit_evac_g(k)
                    vcsb[k] = emit_evac_vc(k)
                    dd[k] = emit_dots(k)
                    sig[k] = emit_sig(k)
                    emit_comb(k, last=last)
                    if next_views is not None:
                        next_dxs.append(emit_sub(next_views[k][0]))
                return next_dxs
            for k in range(GRP + coff):
                if k < GRP:
                    mm[k] = emit_mms(k)
                    if not ab_no_logits:
                        gsb[k] = emit_evac_g(k)
                    vcsb[k] = emit_evac_vc(k)
                if 0 <= k - 1 < GRP and not ab_no_logits:
                    dd[k - 1] = emit_dots(k - 1)
                    sig[k - 1] = emit_sig(k - 1)
                if 0 <= k - coff < GRP:
                    emit_comb(k - coff, last=last)
                if next_views is not None and k < GRP:
                    next_dxs.append(emit_sub(next_views[k][0]))
            return next_dxs

        # startup: first tile's data, then A^T (needed by the first matmul),
        # then W~v, then the rest of group 0 — HWDGE is serial at ~625ns/DMA
        # so the issue order is the time-to-first-matmul
        t0xt = pin.tile([128, XT_COLS], BF16, tag="xt0")
        nc.sync.dma_start(t0xt[0:XT_ROWS, :], xt_d[0:XT_ROWS, 0:XT_COLS])
        t0x8 = pin.tile([128, X8_COLS], x8_dt, tag="x80")
        nc.sync.dma_start(t0x8[:], x8_d[0:128, 0:X8_COLS])
        nc.scalar.dma_start(att[:], wat_d[:, :])
        nc.scalar.dma_start(wvt[:], wwv_d[:, :])
        rxt = pin.tile([128, (GRP - 1) * XT_COLS], BF16, tag="xtr")
        nc.sync.dma_start(rxt[0:XT_ROWS, :],
                          xt_d[0:XT_ROWS, XT_COLS:GRP * XT_COLS])
        rx8 = pin.tile([128, (GRP - 1) * X8_COLS], x8_dt, tag="x8r")
        nc.sync.dma_start(rx8[:], x8_d[0:128, X8_COLS:GRP * X8_COLS])
        views = [(t0xt[:], t0x8[:])]
        views += [(rxt[:, i * XT_COLS:(i + 1) * XT_COLS],
                   rx8[:, i * X8_COLS:(i + 1) * X8_COLS])
                  for i in range(GRP - 1)]
        # first-group subs on DVE: it is idle during pipeline fill and
        # ~2.5x faster per sub than POOL
        dxs = emit_subs(views, eng=nc.vector)
        for git in range(NG):
            nxt = git + 1
            if nxt < NG:
                next_views = fetch_group(nxt % NGROUPS)
            else:
                next_views = None
            next_dxs = emit_group(git % NGROUPS, views, dxs, next_views)
            views, dxs = next_views, next_dxs

    spill_excess_waits(nc)
    return nc


def host_pack(inputs):
    """Marshal full inputs into per-core device feeds.

    Returns dict name -> np.ndarray [N_CORES, rows, cols]."""
    import ml_dtypes
    bf = ml_dtypes.bfloat16
    f8 = ml_dtypes.float8_e4m3

    xw = np.asarray(inputs["word_vectors"], np.float32).reshape(B * W, D_IN)
    xc = np.asarray(inputs["char_vectors"], np.float32).reshape(B * W, D_IN)
    ones = np.ones((B * W, 1), np.float32)
    xw_a = np.concatenate([xw, ones], axis=1)        # [B*W, 301]
    xc_a = np.concatenate([xc, ones], axis=1)

    # augmented weights and A^T = W~k W~q^T / temp (host-side weight prep)
    def aug(wn, bn):
        return np.vstack([np.asarray(inputs[wn], np.float32),
                          np.asarray(inputs[bn], np.float32).reshape(1, H)])
    Wq, Wk, Wv = aug("Wq", "bq"), aug("Wk", "bk"), aug("Wv", "bv")
    AT = (Wk @ Wq.T) / TEMP
    wat = np.zeros((128, 3 * 304), np.float32)
    wwv = np.zeros((128, 3 * 512), np.float32)
    for c, (off, sz) in enumerate(CHUNKS):
        wat[0:sz, c * 304:c * 304 + DA] = AT[off:off + sz, :]
        wwv[0:sz, c * 512:(c + 1) * 512] = Wv[off:off + sz, :]
    wat8 = np.ascontiguousarray(wat.astype(bf))
    wwv8 = np.ascontiguousarray(wwv.astype(bf))

    # [cores, tiles, 128 tok, 301] and feature-major transposes
    xwt = xw_a.reshape(N_CORES, TILES, 128, DA)
    xct = xc_a.reshape(N_CORES, TILES, 128, DA)
    xwT = np.swapaxes(xwt, 2, 3)                     # [c, t, 301, 128]
    xcT = np.swapaxes(xct, 2, 3)

    # feature-major bf16 feed (rows 99/100 of chunk 2 zero-padded only)
    xTb = np.zeros((N_CORES, TILES, XT_ROWS, XT_COLS), np.float32)
    for c, (off, sz) in enumerate(CHUNKS):
        xTb[:, :, 0:sz, c * 128:c * 128 + 128] = xwT[:, :, off:off + sz, :]
        xTb[:, :, 0:sz, 384 + c * 128:384 + c * 128 + 128] = \
            xcT[:, :, off:off + sz, :]
    xTb = xTb.astype(bf).reshape(N_CORES, NGROUPS, GRP, XT_ROWS, XT_COLS)
    xTb = np.ascontiguousarray(np.swapaxes(xTb, 2, 3)).reshape(
        N_CORES, NGROUPS * XT_ROWS, GRP * XT_COLS)

    # token-major fp8 feed: [x~w 301B | pad | x~c 301B | pad]
    x8b = np.zeros((N_CORES, TILES, 128, X8_COLS), np.float32)
    x8b[..., 0:DA] = xwt
    x8b[..., 304:304 + DA] = xct
    x8b = x8b.astype(f8).reshape(N_CORES, NGROUPS, GRP, 128, X8_COLS)
    x8b = np.ascontiguousarray(np.swapaxes(x8b, 2, 3)).reshape(
        N_CORES, NGROUPS * 128, GRP * X8_COLS)

    return {
        "xt": xTb,
        "x8": x8b,
        "Wat": np.broadcast_to(wat8, (N_CORES, 128, 3 * 304)).copy(),
        "Wwv": np.broadcast_to(wwv8, (N_CORES, 128, 3 * 512)).copy(),
    }


_CACHED = {}


def kernel(**inputs):
    if "nc" not in _CACHED:
        _CACHED["nc"] = build_program()
    nc = _CACHED["nc"]

    feeds = host_pack(inputs)
    in_maps = [{k: v[c] for k, v in feeds.items()} for c in range(N_CORES)]

    res = run_bass_kernel_spmd(nc, in_maps, list(range(N_CORES)))
    out = np.concatenate(
        [np.asarray(res.results[c]["out"]).astype(np.float32)
         .reshape(BPC, W, 2 * H) for c in range(N_CORES)],
        axis=0,
    )
    return out



# revision 10
# speedup vs baseline: 1.9668x; 1.9668x over previous
"""Trainium2 Bass kernel v2 for nn_AttentionBetweenWordsAndChars.

Math (exact): out_q = Vm + (sigmoid(d_q) - 1/2) * Dv,  q in {word, char}
  Vm = xm @ Wv, Dv = dx @ Wv, xm = (xw+xc)/2, dx = xw-xc (bias-augmented),
  d_w = xw.g, d_c = xc.g -> d_q = dm +- dd/2 with dm = xm.g, dd = dx.g,
  g = dx @ A, A = Wk Wq^T / sqrt(H).

Precision (host-emulated rel-err ~1.0%, tolerance 2e-2):
  - all matmuls fp8-e4m3 with DoubleRow (2 contraction rows/cycle).
  - Vm in 3 residual terms (xm_h.Wh + xm_h.Wl + xm_l.Wh), psum scale x64.
  - Dv 2 terms (dx.Wh + dx.Wl) x64;  g 2 terms (dx.As + dx.Al) x256.
  - weights pre-scaled host-side (Wv x64, A x256) to clear fp8 subnormals;
    output carries x64, divided out on host.

Why this shape: on this tunneled target, read-DMAs cost ~110ns per
descriptor row + ~10us per DMA; the whole per-pass input is therefore
fetched as ONE [128, 60416] fp8 DMA (~36us) into a double-buffered
resident SBUF block, while output writes are cheap (SWDGE, per tile).
Compute sits at ~30-37us/pass: PE 10 DoubleRow matmuls/tile, ACT evacs
+ sigmoid, POOL dot-products + scalar prep, DVE dot-reduces + the two
combine STTs (the structural DVE floor).
"""

import sys

for _p in ("/opt/trn_rl_repo", "/root/.axon_site/_ro/trn_rl_repo"):
    if _p not in sys.path:
        sys.path.insert(0, _p)

import numpy as np

import concourse.bass as bass
import concourse.tile as tile
from concourse import mybir
from concourse.bass_utils import run_bass_kernel_spmd
import bass_rust

B, W, D_IN, H = 64, 512, 300, 512
N_CORES = 8
BPC = B // N_CORES
TOK = BPC * W                     # 4096 tokens per core
TILES = TOK // 128                # 32
DA = D_IN + 1                     # 301 augmented features
TEMP = float(np.sqrt(np.float32(H)))
F32 = mybir.dt.float32
BF16 = mybir.dt.bfloat16
FP8 = mybir.dt.float8e4
AF = mybir.ActivationFunctionType
OP = mybir.AluOpType
DR = mybir.MatmulPerfMode.DoubleRow

# per-tile feed layout (bytes per partition), all fp8
XMH_OFF, XML_OFF, DX_OFF = 0, 256, 512     # [128,2,128] feature-major pairs
T1_OFF = 768    # rows 0-44: xmh feats 256-300 ; rows 45-89: xml feats 256-300
T1B_OFF = 896   # rows 0-44: xmh feats 256-300 (dup for Vm-mm4 half B)
T2_OFF = 1024   # rows 0-44: dx feats 256-300
T2B_OFF = 1152  # dup of T2 (half B of the tail DoubleRows)
XM8_OFF = 1280  # token-major xm [128, 301]
DX8_OFF = 1584  # token-major dx [128, 301]
TILE_STRIDE = 1888
PASS_COLS = TILES * TILE_STRIDE   # 60416 bytes/partition

VSCALE = 64.0                     # value-path psum scale
GSCALE = 256.0                    # logits-path psum scale


def spill_excess_waits(nc, max_keep=1, ev_cap=2):
    """walrus accepts very few sync-wait commands per datapath opcode;
    move excess waits onto pure-wait EventSemaphore instructions."""
    counter = 0
    for f in nc.m.functions:
        for blk in f.blocks:
            insts = blk.instructions
            i = 0
            while i < len(insts):
                inst = insts[i]
                si = inst.sync_info
                if si is None:
                    i += 1
                    continue
                w = list(si.on_wait or [])
                if len(w) > max_keep:
                    spill = w[:-max_keep]
                    for j in range(0, len(spill), ev_cap):
                        ev = mybir.InstEventSemaphore(name=f"wspill_{counter}")
                        counter += 1
                        ev.engine = inst.engine
                        ev.sync_info = bass_rust.SyncInfo(
                            on_wait=spill[j:j + ev_cap], on_update=[]
                        )
                        insts.insert(i, ev)
                        i += 1
                    inst.sync_info.on_wait = w[-max_keep:]
                i += 1
    return counter


def build_program(loop_reps=1, tiles=TILES, ab_no_dma_in=False,
                  ab_no_dma_out=False, out_on_sync=True,
                  ab_no_logits=False, ab_no_combs=False, ab_mm_only=False,
                  ab_fetch_only=False, fetch_split=1, ab_pe_n128=False,
                  ab_contig=False):
    nc = bass.Bass("TRN2", target_bir_lowering=False, debug=False,
                   num_devices=N_CORES)
    xin_d = nc.dram_tensor("xin", [128, tiles * TILE_STRIDE], FP8,
                           kind="ExternalInput").ap()
    wg_d = nc.dram_tensor("wg", [128, 3 * 608], FP8,
                          kind="ExternalInput").ap()
    wv_d = nc.dram_tensor("wv", [128, 6 * 1024], FP8,
                          kind="ExternalInput").ap()
    out_d = nc.dram_tensor("out", [tiles * 128, 2 * H], BF16,
                           kind="ExternalOutput").ap()

    from contextlib import ExitStack

    with tile.TileContext(nc) as tc, ExitStack() as es:
        cpool = es.enter_context(tc.tile_pool(name="consts", bufs=1))

        halfc = cpool.tile([128, 1], F32, tag="halfc")
        nc.gpsimd.memset(halfc[:], 0.5)
        zbias = cpool.tile([128, 1], F32, tag="zbias")
        nc.gpsimd.memset(zbias[:], 0.0)
        # warm the sigmoid table before the loop
        sigwarm = cpool.tile([128, 1], F32, tag="sigwarm")
        nc.scalar.activation(sigwarm[:], zbias[:], AF.Sigmoid, bias=zbias[:])

        wg = cpool.tile([128, 3 * 608], FP8, tag="wg", name="wg")
        wv = cpool.tile([128, 6 * 1024], FP8, tag="wv", name="wv")
        nc.scalar.dma_start(wg[:], wg_d[:, :])
        nc.scalar.dma_start(wv[:], wv_d[:, :])
        # moving-operand views [K, 2, N]
        mv_as01 = wg[0:128, 0:608].rearrange("p (two n) -> p two n", two=2)
        mv_al01 = wg[0:128, 608:1216].rearrange("p (two n) -> p two n", two=2)
        mv_tail_g = wg[0:45, 1216:1824].rearrange("p (two n) -> p two n",
                                                  two=2)
        mv_wh01 = wv[0:128, 0:1024].rearrange("p (two n) -> p two n", two=2)
        mv_wl01 = wv[0:128, 1024:2048].rearrange("p (two n) -> p two n",
                                                 two=2)
        mv_vm4 = wv[0:90, 2048:3072].rearrange("p (two n) -> p two n", two=2)
        mv_dv3 = wv[0:45, 3072:4096].rearrange("p (two n) -> p two n", two=2)
        # Dv movers at half scale (x32): out = Vm + tanh(d/2) * (Dv/2)
        mv_dh01 = wv[0:128, 4096:5120].rearrange("p (two n) -> p two n",
                                                 two=2)
        mv_dl01 = wv[0:128, 5120:6144].rearrange("p (two n) -> p two n",
                                                 two=2)

        pin = es.enter_context(tc.tile_pool(name="pin", bufs=2))
        ps_g = es.enter_context(tc.tile_pool(name="ps_g", bufs=2,
                                             space="PSUM"))
        ps_vm = es.enter_context(tc.tile_pool(name="ps_vm", bufs=3,
                                              space="PSUM"))
        ps_dv = es.enter_context(tc.tile_pool(name="ps_dv", bufs=2,
                                              space="PSUM"))
        pgsb = es.enter_context(tc.tile_pool(name="pgsb", bufs=3))
        pdvs = es.enter_context(tc.tile_pool(name="pdvs", bufs=4))
        ppm = es.enter_context(tc.tile_pool(name="ppm", bufs=6))
        psm = es.enter_context(tc.tile_pool(name="psm", bufs=10))
        pout = es.enter_context(tc.tile_pool(name="pout", bufs=4))

        outq = nc.sync if out_on_sync else nc.gpsimd

        def emit_pass(xb, first):
            st = {}

            def stat(t, off, k, w):
                base = t * TILE_STRIDE
                return xb[0:k, base + off:base + off + 2 * w].rearrange(
                    "p (two m) -> p two m", two=2)

            def mms(t):
                g_ps = ps_g.tile([128, 304], F32, tag="g")
                vm_ps = ps_vm.tile([128, H], F32, tag="vm")
                dv_ps = ps_dv.tile([128, H], F32, tag="dv")
                dx01 = stat(t, DX_OFF, 128, 128)
                xmh01 = stat(t, XMH_OFF, 128, 128)
                xml01 = stat(t, XML_OFF, 128, 128)
                tail1 = stat(t, T1_OFF, 90, 128)
                tail2 = stat(t, T2_OFF, 45, 128)
                if ab_pe_n128:
                    # same instruction count, N=128 outputs: separates
                    # LDWEIGHTS-bound from N-cycle-bound PE time
                    g_ps128 = g_ps[:, 0:64]
                    v128 = vm_ps[:, 0:64]
                    d128 = dv_ps[:, 0:64]
                    m128 = [(g_ps128, dx01, mv_as01), (g_ps128, dx01, mv_al01),
                            (d128, dx01, mv_dh01), (d128, dx01, mv_dl01),
                            (g_ps128, tail2, mv_tail_g), (d128, tail2, mv_dv3),
                            (v128, xmh01, mv_wh01), (v128, xmh01, mv_wl01),
                            (v128, xml01, mv_wh01), (v128, tail1, mv_vm4)]
                    for i, (o, l, m) in enumerate(m128):
                        nc.tensor.matmul(o, l, m[:, :, 0:64],
                                         start=(i in (0, 2, 6)),
                                         stop=(i in (4, 5, 9)), perf_mode=DR)
                    st[t] = {"g": g_ps, "vm": vm_ps, "dv": dv_ps}
                    return
                if ab_contig:
                    nc.tensor.matmul(g_ps[:], dx01, mv_as01, start=True,
                                     stop=False, perf_mode=DR)
                    nc.tensor.matmul(g_ps[:], dx01, mv_al01, start=False,
                                     stop=False, perf_mode=DR)
                    nc.tensor.matmul(g_ps[:], tail2, mv_tail_g, start=False,
                                     stop=True, perf_mode=DR)
                    nc.tensor.matmul(dv_ps[:], dx01, mv_dh01, start=True,
                                     stop=False, perf_mode=DR)
                    nc.tensor.matmul(dv_ps[:], dx01, mv_dl01, start=False,
                                     stop=False, perf_mode=DR)
                    nc.tensor.matmul(dv_ps[:], tail2, mv_dv3, start=False,
                                     stop=True, perf_mode=DR)
                else:
                    nc.tensor.matmul(g_ps[:], dx01, mv_as01, start=True,
                                     stop=False, perf_mode=DR)
                    nc.tensor.matmul(g_ps[:], dx01, mv_al01, start=False,
                                     stop=False, perf_mode=DR)
                    nc.tensor.matmul(dv_ps[:], dx01, mv_dh01, start=True,
                                     stop=False, perf_mode=DR)
                    nc.tensor.matmul(dv_ps[:], dx01, mv_dl01, start=False,
                                     stop=False, perf_mode=DR)
                    nc.tensor.matmul(g_ps[:], tail2, mv_tail_g, start=False,
                                     stop=True, perf_mode=DR)
                    nc.tensor.matmul(dv_ps[:], tail2, mv_dv3, start=False,
                                     stop=True, perf_mode=DR)
                nc.tensor.matmul(vm_ps[:], xmh01, mv_wh01, start=True,
                                 stop=False, perf_mode=DR)
                nc.tensor.matmul(vm_ps[:], xmh01, mv_wl01, start=False,
                                 stop=False, perf_mode=DR)
                nc.tensor.matmul(vm_ps[:], xml01, mv_wh01, start=False,
                                 stop=False, perf_mode=DR)
                nc.tensor.matmul(vm_ps[:], tail1, mv_vm4, start=False,
                                 stop=True, perf_mode=DR)
                st[t] = {"g": g_ps, "vm": vm_ps, "dv": dv_ps}

            def evac_dots(t):
                s = st[t]
                dv_sb = pdvs.tile([128, H], BF16, tag="dvsb")
                nc.scalar.copy(dv_sb[:], s["dv"][:])
                s["dvsb"] = dv_sb
                base = t * TILE_STRIDE
                xm8 = xb[0:128, base + XM8_OFF:base + XM8_OFF + DA]
                dxh8 = xb[0:128, base + DX8_OFF:base + DX8_OFF + DA]
                dm = psm.tile([128, 2], F32, tag="dm")
                junk = ppm.tile([128, 304], BF16, tag="junk")
                # dm = sum(g * xm8), ddh = sum(g * dx8/2); g carries x256
                nc.vector.scalar_tensor_tensor(
                    out=junk[:, 0:DA], in0=s["g"][:, 0:DA], scalar=1.0,
                    in1=xm8, op0=OP.mult, op1=OP.mult,
                    accum_out=dm[:, 0:1])
                junk2 = ppm.tile([128, 304], BF16, tag="junk2")
                nc.vector.scalar_tensor_tensor(
                    out=junk2[:, 0:DA], in0=s["g"][:, 0:DA], scalar=1.0,
                    in1=dxh8, op0=OP.mult, op1=OP.mult,
                    accum_out=dm[:, 1:2])
                s["dm"] = dm

            def tanhs(t):
                s = st[t]
                dm = s["dm"]
                dm512 = psm.tile([128, 1], F32, tag="dm512")
                nc.scalar.mul(dm512[:], dm[:, 0:1], 1.0 / 512)
                tt = psm.tile([128, 2], F32, tag="tt")
                # t_q = tanh(d_q / 2); d_q/2 = (dm +- ddh)/512
                nc.scalar.activation(tt[:, 0:1], dm[:, 1:2], AF.Tanh,
                                     bias=dm512[:], scale=1.0 / 512)
                nc.scalar.activation(tt[:, 1:2], dm[:, 1:2], AF.Tanh,
                                     bias=dm512[:], scale=-1.0 / 512)
                dt10 = psm.tile([128, 1], F32, tag="dt10")
                nc.scalar.activation(dt10[:], tt[:, 0:1], AF.Identity,
                                     bias=tt[:, 1:2], scale=-1.0)
                s["tt"], s["dt10"] = tt, dt10

            def combs(t):
                s = st[t]
                outt = pout.tile([128, 2 * H], BF16, tag="out")
                sc0 = halfc[:, 0:1] if ab_no_logits else s["tt"][:, 0:1]
                sc1 = zbias[:, 0:1] if ab_no_logits else s["dt10"][:, 0:1]
                nc.vector.scalar_tensor_tensor(
                    out=outt[:, 0:H], in0=s["dvsb"][:], scalar=sc0,
                    in1=s["vm"][:], op0=OP.mult, op1=OP.add)
                nc.vector.scalar_tensor_tensor(
                    out=outt[:, H:2 * H], in0=s["dvsb"][:],
                    scalar=sc1, in1=outt[:, 0:H],
                    op0=OP.mult, op1=OP.add)
                s["out"] = outt

            def outdma(t):
                if ab_no_dma_out:
                    st.pop(t)
                    return
                r0 = t * 128
                outq.dma_start(out_d[r0:r0 + 128, :], st[t]["out"][:])
                st.pop(t)

            do_logits = not (ab_no_logits or ab_mm_only)
            do_combs = not (ab_no_combs or ab_mm_only)
            for t in range(tiles + 3):
                if t < tiles:
                    mms(t)
                if 0 <= t - 2 < tiles and do_logits:
                    tanhs(t - 2)
                if 0 <= t - 1 < tiles:
                    if do_logits:
                        evac_dots(t - 1)
                    elif do_combs:
                        s = st[t - 1]
                        dv_sb = pdvs.tile([128, H], BF16, tag="dvsb")
                        nc.scalar.copy(dv_sb[:], s["dv"][:])
                        s["dvsb"] = dv_sb
                if 0 <= t - 2 < tiles and do_combs:
                    combs(t - 2)
                if 0 <= t - 3 < tiles and do_combs:
                    outdma(t - 3)

        # the fetch rides the ACT HWDGE ring (one DMA/pass) so the per-tile
        # output DMAs on the SP ring are never stuck behind its ~30us
        # transfer (rings are FIFO per issuing engine). The next pass's
        # fetch is emitted BEFORE this pass's compute so its transfer
        # overlaps the whole pass.
        def fetch():
            xb = pin.tile([128, tiles * TILE_STRIDE], FP8, tag="xb")
            if ab_no_dma_in:
                return xb
            cols = tiles * TILE_STRIDE
            if fetch_split <= 1:
                nc.scalar.dma_start(xb[:], xin_d[:, :])
            else:
                qs = [nc.scalar, nc.sync, nc.gpsimd][:fetch_split]
                step = (cols // fetch_split + 127) & ~127
                for i, q in enumerate(qs):
                    lo = i * step
                    hi = min(cols, lo + step)
                    if lo < hi:
                        q.dma_start(xb[:, lo:hi], xin_d[:, lo:hi])
            return xb

        if ab_fetch_only:
            for r in range(loop_reps):
                fetch()
        else:
            xb_cur = fetch()
            for r in range(loop_reps):
                xb_next = fetch() if r + 1 < loop_reps else None
                emit_pass(xb_cur, r == 0)
                xb_cur = xb_next

    spill_excess_waits(nc)
    return nc


def host_pack(inputs, tiles=TILES):
    """Marshal full inputs into per-core mega feeds + fp8 weight packs."""
    import ml_dtypes
    f8 = ml_dtypes.float8_e4m3

    ntok = tiles * 128

    xw = np.asarray(inputs["word_vectors"], np.float32).reshape(B * W, D_IN)
    xc = np.asarray(inputs["char_vectors"], np.float32).reshape(B * W, D_IN)
    ones = np.ones((B * W, 1), np.float32)
    xw = np.concatenate([xw, ones], 1)
    xc = np.concatenate([xc, ones], 1)
    xm = (xw + xc) * 0.5
    dx = xw - xc

    def aug(wn, bn):
        return np.vstack([np.asarray(inputs[wn], np.float32),
                          np.asarray(inputs[bn], np.float32).reshape(1, H)])

    Wq, Wk, Wv = aug("Wq", "bq"), aug("Wk", "bk"), aug("Wv", "bv")
    A = (Wk @ Wq.T) / TEMP                       # g = dx @ A

    # fp8 weight packs (scaled)
    As = (A * GSCALE).astype(f8)
    Al = (A * GSCALE - As.astype(np.float32)).astype(f8)
    Wh = (Wv * VSCALE).astype(f8)
    Wl = (Wv * VSCALE - Wh.astype(np.float32)).astype(f8)

    Wh32 = (Wv * VSCALE * 0.5).astype(f8)
    Wl32 = (Wv * VSCALE * 0.5 - Wh32.astype(np.float32)).astype(f8)

    wg = np.zeros((128, 3 * 608), f8)
    wv = np.zeros((128, 6 * 1024), f8)
    # g movers: (As0|As1), (Al0|Al1), tail (As2|Al2); N=304 (301 valid)
    wg[:, 0:DA] = As[0:128]
    wg[:, 304:304 + DA] = As[128:256]
    wg[:, 608:608 + DA] = Al[0:128]
    wg[:, 912:912 + DA] = Al[128:256]
    wg[0:45, 1216:1216 + DA] = As[256:301]
    wg[0:45, 1520:1520 + DA] = Al[256:301]
    # value movers: (Wh0|Wh1), (Wl0|Wl1), vm4 (Wh2;Wh2 | Wl2;0), dv3 (Wh2|Wl2)
    wv[:, 0:512] = Wh[0:128]
    wv[:, 512:1024] = Wh[128:256]
    wv[:, 1024:1536] = Wl[0:128]
    wv[:, 1536:2048] = Wl[128:256]
    wv[0:45, 2048:2560] = Wh[256:301]
    wv[45:90, 2048:2560] = Wh[256:301]
    wv[0:45, 2560:3072] = Wl[256:301]            # rows 45-89 stay zero
    wv[0:45, 3072:3584] = Wh32[256:301]          # Dv tail, x32 scale
    wv[0:45, 3584:4096] = Wl32[256:301]
    wv[:, 4096:4608] = Wh32[0:128]               # Dv movers, x32 scale
    wv[:, 4608:5120] = Wh32[128:256]
    wv[:, 5120:5632] = Wl32[0:128]
    wv[:, 5632:6144] = Wl32[128:256]

    # per-core token tensors [C, tiles, 128, 301]
    def percore(x):
        return x.reshape(N_CORES, TILES, 128, DA)[:, :tiles]

    xm_t = percore(xm)
    dx_t = percore(dx)
    xmh = xm_t.astype(f8)
    xml = (xm_t - xmh.astype(np.float32)).astype(f8)
    dx8 = dx_t.astype(f8)

    # feature-major [C, tiles, 301, 128]
    xmhT = np.swapaxes(xmh, 2, 3)
    xmlT = np.swapaxes(xml, 2, 3)
    dxT = np.swapaxes(dx8, 2, 3)

    mega = np.zeros((N_CORES, tiles, 128, TILE_STRIDE), f8)
    mega[:, :, :, XMH_OFF:XMH_OFF + 128] = xmhT[:, :, 0:128]
    mega[:, :, :, XMH_OFF + 128:XMH_OFF + 256] = xmhT[:, :, 128:256]
    mega[:, :, :, XML_OFF:XML_OFF + 128] = xmlT[:, :, 0:128]
    mega[:, :, :, XML_OFF + 128:XML_OFF + 256] = xmlT[:, :, 128:256]
    mega[:, :, :, DX_OFF:DX_OFF + 128] = dxT[:, :, 0:128]
    mega[:, :, :, DX_OFF + 128:DX_OFF + 256] = dxT[:, :, 128:256]
    mega[:, :, 0:45, T1_OFF:T1_OFF + 128] = xmhT[:, :, 256:301]
    mega[:, :, 45:90, T1_OFF:T1_OFF + 128] = xmlT[:, :, 256:301]
    mega[:, :, 0:45, T1B_OFF:T1B_OFF + 128] = xmhT[:, :, 256:301]
    mega[:, :, 0:45, T2_OFF:T2_OFF + 128] = dxT[:, :, 256:301]
    mega[:, :, 0:45, T2B_OFF:T2B_OFF + 128] = dxT[:, :, 256:301]
    mega[:, :, :, XM8_OFF:XM8_OFF + DA] = xmh  # token-major (fp8 of xm)
    mega[:, :, :, DX8_OFF:DX8_OFF + DA] = (dx_t * 0.5).astype(f8)
    # [C, 128, tiles*stride]
    mega = np.ascontiguousarray(np.swapaxes(mega, 1, 2)).reshape(
        N_CORES, 128, tiles * TILE_STRIDE)

    return {
        "xin": mega,
        "wg": np.broadcast_to(wg, (N_CORES, 128, 3 * 608)).copy(),
        "wv": np.broadcast_to(wv, (N_CORES, 128, 6 * 1024)).copy(),
    }


_CACHED = {}


def kernel(**inputs):
    if "nc" not in _CACHED:
        _CACHED["nc"] = build_program()
    nc = _CACHED["nc"]

    feeds = host_pack(inputs)
    in_maps = [{k: v[c] for k, v in feeds.items()} for c in range(N_CORES)]

    res = run_bass_kernel_spmd(nc, in_maps, list(range(N_CORES)))
    out = np.concatenate(
        [np.asarray(res.results[c]["out"]).astype(np.float32)
         .reshape(BPC, W, 2 * H) for c in range(N_CORES)],
        axis=0,
    ) * (1.0 / VSCALE)
    return out
